# revision 30
# baseline (speedup 1.0000x reference)
"""ChebConv(K=3) + BatchNorm1d GNN kernel for 8 Trainium2 NeuronCores.

Strategy (graph/data parallel, destination-sharded):
  - Nodes padded to 50176 and split into 8 chunks of 6272 (49 blocks of 128).
  - Edges bucketed by destination owner; each core aggregates only edges whose
    destination it owns.  Source features are gathered with `dma_gather` from a
    full local copy of the (dis-scaled, 512B-row-padded) feature table.
  - Per 128-edge tile, a one-hot selection matrix S[e,d] = (col_local[e]==d) is
    built on DVE (iota + is_equal) and the segment sum is S.T @ V on the PE,
    accumulated in PSUM per 128-destination block.
  - Chebyshev: T0=x, T1=prop(x), T2=2*prop(T1)-x with prop folded as
    prop(h)[c] = -dis[c] * sum_e (dis[row_e] * h[row_e]).  The dis[row] factor
    is folded into the gather table (g = dis * h), so no per-edge weights.
  - Between hops the per-core T1 chunks are AllGathered into the hop-2 table.
  - out^T = sum_k W_k^T @ T_k^T per block (PE transposes + matmuls), BatchNorm
    statistics are partial-summed per core and AllReduced (2x96 floats).
  - Index split: dma_gather indices are int16, so the table is addressed as
    two halves of 25088 rows.
"""
import numpy as np

N = 50000
E = 800000
D = 96
K = 3
EPS = 1e-5
NCORES = 8
CHUNK = 6272            # nodes per core (49 * 128)
NBLK = CHUNK // 128     # 49
NPAD = NCORES * CHUNK   # 50176
HALF = NPAD // 2        # 25088 (< int16 max)
NXBLK = NPAD // 128     # 392
ES = 128                # table row elements (512B rows)
CHUNK_TILES = 8         # tiles (of 128 edges) per dma_gather call
                        # (1024 idxs — the HW ucode cap per dma_gather)
SIM_SINGLE = False      # stub collectives with local DMAs (timeline sim only)

_cache = {}


def _preprocess(x, edge_index):
    row = np.asarray(edge_index[0]).astype(np.int64)
    col = np.asarray(edge_index[1]).astype(np.int64)
    keep = row != col
    row, col = row[keep], col[keep]
    deg = np.bincount(row, minlength=N).astype(np.float32)

    owner = col // CHUNK
    halfv = (row >= HALF).astype(np.int64)
    blk = (col % CHUNK) // 128
    cloc = (col % CHUNK) % 128

    counts = np.zeros((NCORES, NBLK, 2), np.int64)
    np.add.at(counts, (owner, blk, halfv), 1)
    T = np.maximum(1, -(-counts.max(axis=0) // 128))  # [NBLK, 2] tiles/block/half

    cap = T * 128
    base = np.zeros((2, NBLK), np.int64)
    base[0, 1:] = np.cumsum(cap[:-1, 0])
    base[1, 1:] = np.cumsum(cap[:-1, 1])
    Llo, Lhi = int(cap[:, 0].sum()), int(cap[:, 1].sum())

    order = np.lexsort((cloc, blk, halfv, owner))
    row_s, owner_s = row[order], owner[order]
    half_s, blk_s, cloc_s = halfv[order], blk[order], cloc[order]

    xpad = np.zeros((NPAD, D), np.float32)
    xpad[:N] = np.asarray(x, np.float32)
    degpad = np.zeros(NPAD, np.float32)
    degpad[:N] = deg

    per_core = []
    for k in range(NCORES):
        sel = owner_s == k
        h_k, b_k, cl_k, r_k = half_s[sel], blk_s[sel], cloc_s[sel], row_s[sel]
        grp = h_k * NBLK + b_k
        if len(grp):
            starts = np.r_[0, np.flatnonzero(np.diff(grp)) + 1]
            lens = np.diff(np.r_[starts, len(grp)])
            rank = np.arange(len(grp)) - np.repeat(starts, lens)
        else:
            rank = np.zeros(0, np.int64)
        pos = base[h_k, b_k] + rank
        idx_lo = np.zeros(Llo, np.int16)
        idx_hi = np.zeros(Lhi, np.int16)
        cl_lo = np.full(Llo, -1.0, np.float32)
        cl_hi = np.full(Lhi, -1.0, np.float32)
        lo = h_k == 0
        idx_lo[pos[lo]] = r_k[lo].astype(np.int16)
        cl_lo[pos[lo]] = cl_k[lo]
        idx_hi[pos[~lo]] = (r_k[~lo] - HALF).astype(np.int16)
        cl_hi[pos[~lo]] = cl_k[~lo]

        def wrap(a):  # [L] -> [128, L/16] (16-wrapped, replicated over 8 groups)
            return np.tile(a.reshape(-1, 16).T, (8, 1)).astype(np.int16)

        colv = np.concatenate(
            [cl_lo.reshape(-1, 128).T, cl_hi.reshape(-1, 128).T], axis=1
        ).astype(np.float32)
        per_core.append({
            "idx_lo": np.ascontiguousarray(wrap(idx_lo)),
            "idx_hi": np.ascontiguousarray(wrap(idx_hi)),
            "colv": np.ascontiguousarray(colv),
            "x_own": np.ascontiguousarray(xpad[k * CHUNK:(k + 1) * CHUNK]),
            "deg_own": np.ascontiguousarray(
                degpad[k * CHUNK:(k + 1) * CHUNK].reshape(NBLK, 128).T),
        })
    deg_t = np.ascontiguousarray(degpad.reshape(NXBLK, 128).T)
    return T, per_core, deg_t, xpad


def _build(T):
    import concourse.bass as bass
    import concourse.bacc as bacc
    import concourse.mybir as mybir
    import concourse.tile as tile
    from concourse.masks import make_identity

    f32 = mybir.dt.float32
    f16 = mybir.dt.float16
    i16 = mybir.dt.int16
    Alu = mybir.AluOpType
    Act = mybir.ActivationFunctionType

    cap = T * 128
    Llo, Lhi = int(cap[:, 0].sum()), int(cap[:, 1].sum())
    ntlo = int(T[:, 0].sum())
    # tile lists per half: (block, first_of_block, last_of_block)
    tiles_h = []
    for h in (0, 1):
        lst = []
        for b in range(NBLK):
            for i in range(int(T[b, h])):
                lst.append((b, i == 0, i == int(T[b, h]) - 1))
        tiles_h.append(lst)

    def chunks(n):
        out, c0 = [], 0
        while c0 < n:
            cn = min(CHUNK_TILES, n - c0)
            out.append((c0, cn))
            c0 += cn
        return out

    nc = bacc.Bacc("TRN2", target_bir_lowering=False, debug=False,
                   num_devices=NCORES, num_swdge_queues=2)
    x_d = nc.dram_tensor("x", [NPAD, D], f32, kind="ExternalInput")
    xo_d = nc.dram_tensor("x_own", [CHUNK, D], f32, kind="ExternalInput")
    degt_d = nc.dram_tensor("deg_t", [128, NXBLK], f32, kind="ExternalInput")
    dego_d = nc.dram_tensor("deg_own", [128, NBLK], f32, kind="ExternalInput")
    il_d = nc.dram_tensor("idx_lo", [128, Llo // 16], i16, kind="ExternalInput")
    ih_d = nc.dram_tensor("idx_hi", [128, Lhi // 16], i16, kind="ExternalInput")
    cv_d = nc.dram_tensor("colv", [128, ntlo + int(T[:, 1].sum())], f32,
                          kind="ExternalInput")
    w_d = nc.dram_tensor("W", [K, D, D], f32, kind="ExternalInput")
    gam_d = nc.dram_tensor("gamma", [D, 1], f32, kind="ExternalInput")
    bet_d = nc.dram_tensor("beta", [D, 1], f32, kind="ExternalInput")
    y_d = nc.dram_tensor("y", [CHUNK, D], f32, kind="ExternalOutput")

    with tile.TileContext(nc) as tc:
        with tc.tile_pool(name="const", bufs=1) as cpool, \
             tc.tile_pool(name="pers", bufs=1) as pers, \
             tc.tile_pool(name="work", bufs=1) as work, \
             tc.tile_pool(name="vpool", bufs=4) as vpool, \
             tc.tile_pool(name="spool", bufs=6) as spool, \
             tc.tile_pool(name="xrot", bufs=4) as xrot, \
             tc.tile_pool(name="psum_seg", bufs=4, space="PSUM") as pseg, \
             tc.tile_pool(name="psum_tp", bufs=2, space="PSUM") as ptp, \
             tc.tile_pool(name="psum_out", bufs=2, space="PSUM") as pout, \
             tc.tile_pool(name="dram", bufs=1, space="DRAM") as dram:

            # ---- constants / persistent loads ----
            ident = cpool.tile([128, 128], f32)
            make_identity(nc, ident[:])
            iota = cpool.tile([128, 128], f16)
            nc.gpsimd.iota(iota[:], pattern=[[1, 128]], base=0,
                           channel_multiplier=0,
                           allow_small_or_imprecise_dtypes=True)
            idxlo = pers.tile([128, Llo // 16], i16)
            idxhi = pers.tile([128, Lhi // 16], i16)
            colv = pers.tile([128, ntlo + int(T[:, 1].sum())], f32)
            nc.sync.dma_start(out=idxlo[:], in_=il_d.ap())
            nc.sync.dma_start(out=idxhi[:], in_=ih_d.ap())
            nc.sync.dma_start(out=colv[:], in_=cv_d.ap())
            w_sb = []
            for k in range(K):
                w_k = pers.tile([D, D], f32, name=f"w{k}", tag=f"w{k}")
                nc.sync.dma_start(out=w_k[:], in_=w_d.ap()[k])
                w_sb.append(w_k)
            gam = cpool.tile([D, 1], f32)
            bet = cpool.tile([D, 1], f32)
            nc.sync.dma_start(out=gam[:], in_=gam_d.ap())
            nc.sync.dma_start(out=bet[:], in_=bet_d.ap())

            # ---- dis tables: dis = (deg>0) / sqrt(max(deg,1)) ----
            def dis_pipeline(src_dram, ncols, nm):
                dg = work.tile([128, ncols], f32, name=f"dg_{nm}", tag=f"dg{nm}")
                nc.sync.dma_start(out=dg[:], in_=src_dram.ap())
                dm = work.tile([128, ncols], f32, name=f"dm_{nm}", tag=f"dm{nm}")
                nc.vector.tensor_scalar(out=dm[:], in0=dg[:], scalar1=1.0,
                                        scalar2=None, op0=Alu.max)
                nc.scalar.sqrt(dm[:], dm[:])
                rec = work.tile([128, ncols], f32, name=f"rec_{nm}", tag=f"rc{nm}")
                nc.vector.reciprocal(rec[:], dm[:])
                msk = work.tile([128, ncols], f32, name=f"msk_{nm}", tag=f"mk{nm}")
                nc.vector.tensor_scalar(out=msk[:], in0=dg[:], scalar1=1.0,
                                        scalar2=None, op0=Alu.is_ge)
                dis = pers.tile([128, ncols], f32, name=f"dis_{nm}", tag=f"ds{nm}")
                nc.vector.tensor_tensor(out=dis[:], in0=rec[:], in1=msk[:],
                                        op=Alu.mult)
                return dis

            dis_full = dis_pipeline(degt_d, NXBLK, "f")
            dis_own = dis_pipeline(dego_d, NBLK, "o")
            ndis_own = pers.tile([128, NBLK], f32)
            nc.vector.tensor_scalar(out=ndis_own[:], in0=dis_own[:],
                                    scalar1=-1.0, scalar2=None, op0=Alu.mult)

            # ---- DRAM tables ----
            g0_full = dram.tile([NPAD, ES], f16, name="g0_full")
            gx_in = dram.tile([CHUNK, ES], f16, name="gx_in")
            g1_full = dram.tile([NPAD, ES], f16, name="g1_full")
            st_in = dram.tile([D, 2], f32, name="st_in")
            st_out = dram.tile([D, 2], f32, name="st_out")

            # rotating staging tiles for table rows (pad cols zeroed once)
            GB = max(d for d in range(1, 17) if NXBLK % d == 0)
            GYB = max(d for d in range(1, 9) if NBLK % d == 0)
            gtiles = []
            for i in range(4):
                g = pers.tile([128, GB, ES], f16, name=f"gt{i}", tag=f"gt{i}")
                nc.vector.memset(g[:, :, D:], 0.0)
                gtiles.append(g)
            g1tiles = []
            for i in range(2):
                g = pers.tile([128, GYB, ES], f16, name=f"g1t{i}",
                              tag=f"g1t{i}")
                nc.vector.memset(g[:, :, D:], 0.0)
                g1tiles.append(g)

            # persistent per-block state
            t1s = [pers.tile([128, D], f32, name=f"t1_{b}", tag=f"t1_{b}")
                   for b in range(NBLK)]
            outT = [pers.tile([D, 128], f32, name=f"oT{b}", tag=f"oT{b}")
                    for b in range(NBLK)]
            s1c = pers.tile([D, NBLK], f32)
            s2c = pers.tile([D, NBLK], f32)

            xown_big = pers.tile([128, NBLK, D], f32)
            nc.sync.dma_start(
                out=xown_big[:],
                in_=xo_d.ap().rearrange("(n p) d -> p n d", p=128))
            xown = [xown_big[:, b, :] for b in range(NBLK)]

            # ---- phase A: hop-1 table g0 = dis * x (full, built locally) ----
            xv = x_d.ap().rearrange("(n p) d -> p n d", p=128)
            g0v = g0_full[:].rearrange("(n p) d -> p n d", p=128)
            for jg in range(0, NXBLK, GB):
                xj = xrot.tile([128, GB, D], f32, name="xj", tag="xj")
                nc.sync.dma_start(out=xj[:], in_=xv[:, jg:jg + GB, :])
                g = gtiles[(jg // GB) % 4]
                for i in range(GB):
                    nc.vector.tensor_scalar(
                        out=g[:, i, :D], in0=xj[:, i, :],
                        scalar1=dis_full[:, jg + i:jg + i + 1],
                        scalar2=None, op0=Alu.mult)
                nc.sync.dma_start(out=g0v[:, jg:jg + GB, :], in_=g[:])

            def phase_E(b, t2_tile):
                """out^T[b] = sum_k W_k^T @ T_k^T; BN partial sums."""
                op = pout.tile([D, 128], f32, name="outps", tag="outps")
                for k, src in enumerate((xown[b], t1s[b], t2_tile)):
                    tp = ptp.tile([D, 128], f32, name="tp", tag="tp")
                    nc.tensor.transpose(out=tp[:], in_=src[:], identity=ident[:])
                    ts = spool.tile([D, 128], f32, name="tT", tag=f"tT{k}")
                    nc.scalar.copy(ts[:], tp[:])
                    nc.tensor.matmul(op[:], lhsT=w_sb[k][:], rhs=ts[:],
                                     start=(k == 0), stop=(k == K - 1))
                nc.scalar.activation(out=outT[b][:], in_=op[:], func=Act.Copy,
                                     accum_out=s1c[:, b:b + 1])
                sq = spool.tile([D, 128], f32, name="sq", tag="sq")
                nc.scalar.activation(out=sq[:], in_=outT[b][:], func=Act.Square,
                                     accum_out=s2c[:, b:b + 1])

            def hop(hop_i, lo_view, hi_view):
                # block-major order: each block's lo tiles then hi tiles feed
                # ONE psum accumulation group; finalize straight from PSUM.
                half_ofs = [0, 0]
                for b in range(NBLK):
                    ntl, nth = int(T[b, 0]), int(T[b, 1])
                    ntot = ntl + nth
                    ps_b = pseg.tile([128, D], f32, name="seg", tag="seg")
                    ti = 0
                    for h, cnt in ((0, ntl), (1, nth)):
                        idx = idxlo if h == 0 else idxhi
                        view = lo_view if h == 0 else hi_view
                        done = 0
                        while done < cnt:
                            cn = min(CHUNK_TILES, cnt - done)
                            c0 = half_ofs[h] + done
                            vb = vpool.tile([128, cn, ES], f16, name="vb",
                                            tag="vb")
                            nc.gpsimd.dma_gather(
                                out_ap=vb[:], in_ap=view,
                                idxs_ap=idx[:, c0 * 8:(c0 + cn) * 8],
                                num_idxs=cn * 128, num_idxs_reg=cn * 128,
                                elem_size=ES, queue_num=(b % 2))
                            for i in range(cn):
                                gt = c0 + i + (0 if h == 0 else ntlo)
                                S = spool.tile([128, 128], f16, name="S",
                                               tag="S")
                                nc.vector.tensor_scalar(
                                    out=S[:], in0=iota[:],
                                    scalar1=colv[:, gt:gt + 1], scalar2=None,
                                    op0=Alu.is_equal)
                                nc.tensor.matmul(ps_b[:], lhsT=S[:],
                                                 rhs=vb[:, i, :D],
                                                 start=(ti == 0),
                                                 stop=(ti == ntot - 1))
                                ti += 1
                            done += cn
                    half_ofs[0] += ntl
                    half_ofs[1] += nth
                    if hop_i == 1:
                        nc.vector.tensor_scalar(
                            out=t1s[b][:], in0=ps_b[:],
                            scalar1=ndis_own[:, b:b + 1],
                            scalar2=None, op0=Alu.mult)
                        g = g1tiles[(b // GYB) % 2]
                        nc.vector.tensor_scalar(
                            out=g[:, b % GYB, :D], in0=t1s[b][:],
                            scalar1=dis_own[:, b:b + 1],
                            scalar2=None, op0=Alu.mult)
                        if b % GYB == GYB - 1:
                            gx_v = gx_in[:].rearrange("(n p) d -> p n d",
                                                      p=128)
                            nc.sync.dma_start(
                                out=gx_v[:, b - GYB + 1:b + 1, :], in_=g[:])
                    else:
                        t2 = xrot.tile([128, D], f32, name="t2", tag="t2")
                        nc.vector.tensor_scalar(
                            out=t2[:], in0=ps_b[:],
                            scalar1=ndis_own[:, b:b + 1],
                            scalar2=2.0, op0=Alu.mult, op1=Alu.mult)
                        nc.vector.tensor_tensor(
                            out=t2[:], in0=t2[:], in1=xown[b][:],
                            op=Alu.subtract)
                        phase_E(b, t2)

            # ---- hop 1 ----
            hop(1, g0_full[0:HALF, :], g0_full[HALF:NPAD, :])
            # ---- exchange T1 ----
            if SIM_SINGLE:
                nc.sync.dma_start(out=g1_full[0:CHUNK, :], in_=gx_in[:])
            else:
                nc.gpsimd.collective_compute(
                    "AllGather", Alu.bypass,
                    replica_groups=[list(range(NCORES))],
                    ins=[gx_in.opt()], outs=[g1_full.opt()])
            # ---- hop 2 (+ phase E per block) ----
            hop(2, g1_full[0:HALF, :], g1_full[HALF:NPAD, :])

            # ---- BN stats reduce + AllReduce ----
            st = work.tile([D, 2], f32)
            nc.vector.tensor_reduce(out=st[:, 0:1], in_=s1c[:],
                                    axis=mybir.AxisListType.X, op=Alu.add)
            nc.vector.tensor_reduce(out=st[:, 1:2], in_=s2c[:],
                                    axis=mybir.AxisListType.X, op=Alu.add)
            nc.sync.dma_start(out=st_in[:], in_=st[:])
            if SIM_SINGLE:
                nc.sync.dma_start(out=st_out[:], in_=st_in[:])
            else:
                nc.gpsimd.collective_compute(
                    "AllReduce", Alu.add,
                    replica_groups=[list(range(NCORES))],
                    ins=[st_in.opt()], outs=[st_out.opt()])
            gst = work.tile([D, 2], f32)
            nc.sync.dma_start(out=gst[:], in_=st_out[:])
            mean = work.tile([D, 1], f32)
            nc.vector.tensor_scalar(out=mean[:], in0=gst[:, 0:1],
                                    scalar1=1.0 / N, scalar2=None, op0=Alu.mult)
            var = work.tile([D, 1], f32)
            nc.vector.tensor_scalar(out=var[:], in0=gst[:, 1:2],
                                    scalar1=1.0 / N, scalar2=None, op0=Alu.mult)
            msq = work.tile([D, 1], f32)
            nc.vector.tensor_tensor(out=msq[:], in0=mean[:], in1=mean[:],
                                    op=Alu.mult)
            nc.vector.tensor_tensor(out=var[:], in0=var[:], in1=msq[:],
                                    op=Alu.subtract)
            nc.vector.tensor_scalar(out=var[:], in0=var[:], scalar1=EPS,
                                    scalar2=None, op0=Alu.add)
            sd = work.tile([D, 1], f32)
            nc.scalar.sqrt(sd[:], var[:])
            inv = work.tile([D, 1], f32)
            nc.vector.reciprocal(inv[:], sd[:])
            scl = work.tile([D, 1], f32)
            nc.vector.tensor_tensor(out=scl[:], in0=gam[:], in1=inv[:],
                                    op=Alu.mult)
            sh = work.tile([D, 1], f32)
            nc.vector.tensor_tensor(out=sh[:], in0=mean[:], in1=scl[:],
                                    op=Alu.mult)
            nc.vector.tensor_tensor(out=sh[:], in0=bet[:], in1=sh[:],
                                    op=Alu.subtract)

            # ---- normalize + emit (batched y writes, 7 blocks per DMA) ----
            yv = y_d.ap().rearrange("(n p) d -> p n d", p=128)
            for b in range(NBLK):
                fin = spool.tile([D, 128], f32, name="fin", tag="fin")
                nc.scalar.activation(out=fin[:], in_=outT[b][:],
                                     func=Act.Identity, bias=sh[:, 0:1],
                                     scale=scl[:, 0:1])
                tp2 = ptp.tile([128, D], f32, name="tp", tag="tp")
                nc.tensor.transpose(out=tp2[:], in_=fin[:],
                                    identity=ident[:D, :D])
                if b % GYB == 0:
                    yb = xrot.tile([128, GYB, D], f32, name="yb", tag="yb")
                nc.scalar.copy(yb[:, b % GYB, :], tp2[:])
                if b % GYB == GYB - 1:
                    nc.sync.dma_start(out=yv[:, b - GYB + 1:b + 1, :],
                                      in_=yb[:])

    nc.compile()
    return nc


def _prepare(x, edge_index, W, gamma, beta):
    x = np.asarray(x, np.float32)
    W = np.asarray(W, np.float32)
    gamma = np.asarray(gamma, np.float32).reshape(D, 1)
    beta = np.asarray(beta, np.float32).reshape(D, 1)

    T, per_core, deg_t, xpad = _preprocess(x, edge_index)

    key = T.tobytes()
    if key not in _cache:
        _cache[key] = _build(T)
    nc = _cache[key]

    in_maps = []
    for k in range(NCORES):
        pc = per_core[k]
        in_maps.append({
            "x": xpad, "x_own": pc["x_own"], "deg_t": deg_t,
            "deg_own": pc["deg_own"], "idx_lo": pc["idx_lo"],
            "idx_hi": pc["idx_hi"], "colv": pc["colv"],
            "W": W, "gamma": gamma, "beta": beta,
        })
    return nc, in_maps


def kernel(x, edge_index, W, bias, gamma, beta):
    from concourse.bass_utils import run_bass_kernel_spmd

    nc, in_maps = _prepare(x, edge_index, W, gamma, beta)
    res = run_bass_kernel_spmd(nc, in_maps, core_ids=list(range(NCORES)))
    y = np.concatenate([res.results[k]["y"] for k in range(NCORES)], axis=0)
    return np.ascontiguousarray(y[:N])


# revision 33
# speedup vs baseline: 1821.0580x; 1821.0580x over previous
"""ChebConv(K=3) + BatchNorm1d GNN kernel for 8 Trainium2 NeuronCores.

Strategy (graph/data parallel, destination-sharded):
  - Nodes padded to 50176 and split into 8 chunks of 6272 (49 blocks of 128).
  - Edges bucketed by destination owner; each core aggregates only edges whose
    destination it owns.  Source features are gathered with `dma_gather` from a
    full local copy of the (dis-scaled, 512B-row-padded) feature table.
  - Per 128-edge tile, a one-hot selection matrix S[e,d] = (col_local[e]==d) is
    built on DVE (iota + is_equal) and the segment sum is S.T @ V on the PE,
    accumulated in PSUM per 128-destination block.
  - Chebyshev: T0=x, T1=prop(x), T2=2*prop(T1)-x with prop folded as
    prop(h)[c] = -dis[c] * sum_e (dis[row_e] * h[row_e]).  The dis[row] factor
    is folded into the gather table (g = dis * h), so no per-edge weights.
  - Between hops the per-core T1 chunks are AllGathered into the hop-2 table.
  - out^T = sum_k W_k^T @ T_k^T per block (PE transposes + matmuls), BatchNorm
    statistics are partial-summed per core and AllReduced (2x96 floats).
  - Index split: dma_gather indices are int16, so the table is addressed as
    two halves of 25088 rows.
"""
import numpy as np

N = 50000
E = 800000
D = 96
K = 3
EPS = 1e-5
NCORES = 8
CHUNK = 6272            # nodes per core (49 * 128)
NBLK = CHUNK // 128     # 49
NPAD = NCORES * CHUNK   # 50176
HALF = NPAD // 2        # 25088 (< int16 max)
NXBLK = NPAD // 128     # 392
ES = 128                # table row elements (512B rows)
CHUNK_TILES = 8         # tiles (of 128 edges) per dma_gather call
                        # (1024 idxs — the HW ucode cap per dma_gather)
SIM_SINGLE = False      # stub collectives with local DMAs (timeline sim only)

_cache = {}


def _preprocess(x, edge_index):
    row = np.asarray(edge_index[0]).astype(np.int64)
    col = np.asarray(edge_index[1]).astype(np.int64)
    keep = row != col
    row, col = row[keep], col[keep]
    deg = np.bincount(row, minlength=N).astype(np.float32)

    owner = col // CHUNK
    halfv = (row >= HALF).astype(np.int64)
    blk = (col % CHUNK) // 128
    cloc = (col % CHUNK) % 128

    counts = np.zeros((NCORES, NBLK, 2), np.int64)
    np.add.at(counts, (owner, blk, halfv), 1)
    T = np.maximum(1, -(-counts.max(axis=0) // 128))  # [NBLK, 2] tiles/block/half

    cap = T * 128
    base = np.zeros((2, NBLK), np.int64)
    base[0, 1:] = np.cumsum(cap[:-1, 0])
    base[1, 1:] = np.cumsum(cap[:-1, 1])
    Llo, Lhi = int(cap[:, 0].sum()), int(cap[:, 1].sum())

    order = np.lexsort((cloc, blk, halfv, owner))
    row_s, owner_s = row[order], owner[order]
    half_s, blk_s, cloc_s = halfv[order], blk[order], cloc[order]

    xpad = np.zeros((NPAD, D), np.float32)
    xpad[:N] = np.asarray(x, np.float32)
    degpad = np.zeros(NPAD, np.float32)
    degpad[:N] = deg

    per_core = []
    for k in range(NCORES):
        sel = owner_s == k
        h_k, b_k, cl_k, r_k = half_s[sel], blk_s[sel], cloc_s[sel], row_s[sel]
        grp = h_k * NBLK + b_k
        if len(grp):
            starts = np.r_[0, np.flatnonzero(np.diff(grp)) + 1]
            lens = np.diff(np.r_[starts, len(grp)])
            rank = np.arange(len(grp)) - np.repeat(starts, lens)
        else:
            rank = np.zeros(0, np.int64)
        pos = base[h_k, b_k] + rank
        idx_lo = np.zeros(Llo, np.int16)
        idx_hi = np.zeros(Lhi, np.int16)
        cl_lo = np.full(Llo, -1.0, np.float32)
        cl_hi = np.full(Lhi, -1.0, np.float32)
        lo = h_k == 0
        idx_lo[pos[lo]] = r_k[lo].astype(np.int16)
        cl_lo[pos[lo]] = cl_k[lo]
        idx_hi[pos[~lo]] = (r_k[~lo] - HALF).astype(np.int16)
        cl_hi[pos[~lo]] = cl_k[~lo]

        def wrap(a):  # [L] -> [128, L/16] (16-wrapped, replicated over 8 groups)
            return np.tile(a.reshape(-1, 16).T, (8, 1)).astype(np.int16)

        colv = np.concatenate(
            [cl_lo.reshape(-1, 128).T, cl_hi.reshape(-1, 128).T], axis=1
        ).astype(np.float32)
        per_core.append({
            "idx_lo": np.ascontiguousarray(wrap(idx_lo)),
            "idx_hi": np.ascontiguousarray(wrap(idx_hi)),
            "colv": np.ascontiguousarray(colv),
            "x_own": np.ascontiguousarray(xpad[k * CHUNK:(k + 1) * CHUNK]),
            "deg_own": np.ascontiguousarray(
                degpad[k * CHUNK:(k + 1) * CHUNK].reshape(NBLK, 128).T),
        })
    deg_t = np.ascontiguousarray(degpad.reshape(NXBLK, 128).T)
    return T, per_core, deg_t, xpad


def _build(T):
    import concourse.bass as bass
    import concourse.bacc as bacc
    import concourse.mybir as mybir
    import concourse.tile as tile
    from concourse.masks import make_identity

    f32 = mybir.dt.float32
    f16 = mybir.dt.float16
    i16 = mybir.dt.int16
    Alu = mybir.AluOpType
    Act = mybir.ActivationFunctionType

    cap = T * 128
    Llo, Lhi = int(cap[:, 0].sum()), int(cap[:, 1].sum())
    ntlo = int(T[:, 0].sum())
    # tile lists per half: (block, first_of_block, last_of_block)
    tiles_h = []
    for h in (0, 1):
        lst = []
        for b in range(NBLK):
            for i in range(int(T[b, h])):
                lst.append((b, i == 0, i == int(T[b, h]) - 1))
        tiles_h.append(lst)

    def chunks(n):
        out, c0 = [], 0
        while c0 < n:
            cn = min(CHUNK_TILES, n - c0)
            out.append((c0, cn))
            c0 += cn
        return out

    nc = bacc.Bacc("TRN2", target_bir_lowering=False, debug=False,
                   num_devices=NCORES, num_swdge_queues=2)
    x_d = nc.dram_tensor("x", [NPAD, D], f32, kind="ExternalInput")
    xo_d = nc.dram_tensor("x_own", [CHUNK, D], f32, kind="ExternalInput")
    degt_d = nc.dram_tensor("deg_t", [128, NXBLK], f32, kind="ExternalInput")
    dego_d = nc.dram_tensor("deg_own", [128, NBLK], f32, kind="ExternalInput")
    il_d = nc.dram_tensor("idx_lo", [128, Llo // 16], i16, kind="ExternalInput")
    ih_d = nc.dram_tensor("idx_hi", [128, Lhi // 16], i16, kind="ExternalInput")
    cv_d = nc.dram_tensor("colv", [128, ntlo + int(T[:, 1].sum())], f32,
                          kind="ExternalInput")
    w_d = nc.dram_tensor("W", [K, D, D], f32, kind="ExternalInput")
    gam_d = nc.dram_tensor("gamma", [D, 1], f32, kind="ExternalInput")
    bet_d = nc.dram_tensor("beta", [D, 1], f32, kind="ExternalInput")
    y_d = nc.dram_tensor("y", [CHUNK, D], f32, kind="ExternalOutput")

    with tile.TileContext(nc) as tc:
        with tc.tile_pool(name="const", bufs=1) as cpool, \
             tc.tile_pool(name="pers", bufs=1) as pers, \
             tc.tile_pool(name="work", bufs=1) as work, \
             tc.tile_pool(name="vpool", bufs=6) as vpool, \
             tc.tile_pool(name="spool", bufs=10) as spool, \
             tc.tile_pool(name="xrot", bufs=4) as xrot, \
             tc.tile_pool(name="psum_seg", bufs=4, space="PSUM") as pseg, \
             tc.tile_pool(name="psum_tp", bufs=2, space="PSUM") as ptp, \
             tc.tile_pool(name="psum_out", bufs=2, space="PSUM") as pout, \
             tc.tile_pool(name="dram", bufs=1, space="DRAM") as dram:

            # ---- constants / persistent loads ----
            ident = cpool.tile([128, 128], f32)
            make_identity(nc, ident[:])
            iota = cpool.tile([128, 128], f16)
            nc.gpsimd.iota(iota[:], pattern=[[1, 128]], base=0,
                           channel_multiplier=0,
                           allow_small_or_imprecise_dtypes=True)
            idxlo = pers.tile([128, Llo // 16], i16)
            idxhi = pers.tile([128, Lhi // 16], i16)
            colv = pers.tile([128, ntlo + int(T[:, 1].sum())], f32)
            nc.sync.dma_start(out=idxlo[:], in_=il_d.ap())
            nc.sync.dma_start(out=idxhi[:], in_=ih_d.ap())
            nc.sync.dma_start(out=colv[:], in_=cv_d.ap())
            w_sb = []
            for k in range(K):
                w_k = pers.tile([D, D], f32, name=f"w{k}", tag=f"w{k}")
                nc.sync.dma_start(out=w_k[:], in_=w_d.ap()[k])
                w_sb.append(w_k)
            gam = cpool.tile([D, 1], f32)
            bet = cpool.tile([D, 1], f32)
            nc.sync.dma_start(out=gam[:], in_=gam_d.ap())
            nc.sync.dma_start(out=bet[:], in_=bet_d.ap())

            # ---- dis tables: dis = (deg>0) / sqrt(max(deg,1)) ----
            def dis_pipeline(src_dram, ncols, nm):
                dg = work.tile([128, ncols], f32, name=f"dg_{nm}", tag=f"dg{nm}")
                nc.sync.dma_start(out=dg[:], in_=src_dram.ap())
                dm = work.tile([128, ncols], f32, name=f"dm_{nm}", tag=f"dm{nm}")
                nc.vector.tensor_scalar(out=dm[:], in0=dg[:], scalar1=1.0,
                                        scalar2=None, op0=Alu.max)
                nc.scalar.sqrt(dm[:], dm[:])
                rec = work.tile([128, ncols], f32, name=f"rec_{nm}", tag=f"rc{nm}")
                nc.vector.reciprocal(rec[:], dm[:])
                msk = work.tile([128, ncols], f32, name=f"msk_{nm}", tag=f"mk{nm}")
                nc.vector.tensor_scalar(out=msk[:], in0=dg[:], scalar1=1.0,
                                        scalar2=None, op0=Alu.is_ge)
                dis = pers.tile([128, ncols], f32, name=f"dis_{nm}", tag=f"ds{nm}")
                nc.vector.tensor_tensor(out=dis[:], in0=rec[:], in1=msk[:],
                                        op=Alu.mult)
                return dis

            dis_full = dis_pipeline(degt_d, NXBLK, "f")
            dis_own = dis_pipeline(dego_d, NBLK, "o")
            ndis_own = pers.tile([128, NBLK], f32)
            nc.vector.tensor_scalar(out=ndis_own[:], in0=dis_own[:],
                                    scalar1=-1.0, scalar2=None, op0=Alu.mult)

            # ---- DRAM tables ----
            g0_full = dram.tile([NPAD, ES], f16, name="g0_full")
            gx_in = dram.tile([CHUNK, ES], f16, name="gx_in")
            g1_full = dram.tile([NPAD, ES], f16, name="g1_full")
            st_in = dram.tile([D, 2], f32, name="st_in")
            st_out = dram.tile([D, 2], f32, name="st_out")

            # rotating staging tiles for table rows (pad cols zeroed once)
            GB = max(d for d in range(1, 17) if NXBLK % d == 0)
            GYB = max(d for d in range(1, 9) if NBLK % d == 0)
            gtiles = []
            for i in range(4):
                g = pers.tile([128, GB, ES], f16, name=f"gt{i}", tag=f"gt{i}")
                nc.vector.memset(g[:, :, D:], 0.0)
                gtiles.append(g)
            g1tiles = []
            for i in range(2):
                g = pers.tile([128, GYB, ES], f16, name=f"g1t{i}",
                              tag=f"g1t{i}")
                nc.vector.memset(g[:, :, D:], 0.0)
                g1tiles.append(g)

            # persistent per-block state
            t1s = [pers.tile([128, D], f32, name=f"t1_{b}", tag=f"t1_{b}")
                   for b in range(NBLK)]
            outT = [pers.tile([D, 128], f32, name=f"oT{b}", tag=f"oT{b}")
                    for b in range(NBLK)]
            s1c = pers.tile([D, NBLK], f32)
            s2c = pers.tile([D, NBLK], f32)

            xown_big = pers.tile([128, NBLK, D], f32)
            nc.sync.dma_start(
                out=xown_big[:],
                in_=xo_d.ap().rearrange("(n p) d -> p n d", p=128))
            xown = [xown_big[:, b, :] for b in range(NBLK)]

            # ---- phase A: hop-1 table g0 = dis * x (full, built locally) ----
            xv = x_d.ap().rearrange("(n p) d -> p n d", p=128)
            g0v = g0_full[:].rearrange("(n p) d -> p n d", p=128)
            for jg in range(0, NXBLK, GB):
                xj = xrot.tile([128, GB, D], f32, name="xj", tag="xj")
                nc.sync.dma_start(out=xj[:], in_=xv[:, jg:jg + GB, :])
                g = gtiles[(jg // GB) % 4]
                for i in range(GB):
                    nc.vector.tensor_scalar(
                        out=g[:, i, :D], in0=xj[:, i, :],
                        scalar1=dis_full[:, jg + i:jg + i + 1],
                        scalar2=None, op0=Alu.mult)
                nc.sync.dma_start(out=g0v[:, jg:jg + GB, :], in_=g[:])

            def phase_E(b, t2_tile):
                """out^T[b] = sum_k W_k^T @ T_k^T; BN partial sums."""
                op = pout.tile([D, 128], f32, name="outps", tag="outps")
                for k, src in enumerate((xown[b], t1s[b], t2_tile)):
                    tp = ptp.tile([D, 128], f32, name="tp", tag="tp")
                    nc.tensor.transpose(out=tp[:], in_=src[:], identity=ident[:])
                    ts = spool.tile([D, 128], f32, name="tT", tag=f"tT{k}")
                    nc.scalar.copy(ts[:], tp[:])
                    nc.tensor.matmul(op[:], lhsT=w_sb[k][:], rhs=ts[:],
                                     start=(k == 0), stop=(k == K - 1))
                nc.scalar.activation(out=outT[b][:], in_=op[:], func=Act.Copy,
                                     accum_out=s1c[:, b:b + 1])
                sq = spool.tile([D, 128], f32, name="sq", tag="sq")
                nc.scalar.activation(out=sq[:], in_=outT[b][:], func=Act.Square,
                                     accum_out=s2c[:, b:b + 1])

            def hop(hop_i, lo_view, hi_view):
                # block-major order: each block's lo tiles then hi tiles feed
                # ONE psum accumulation group; finalize straight from PSUM.
                half_ofs = [0, 0]
                callq = [0]
                for b in range(NBLK):
                    ntl, nth = int(T[b, 0]), int(T[b, 1])
                    ntot = ntl + nth
                    ps_b = pseg.tile([128, D], f32, name="seg", tag="seg")
                    ti = 0
                    for h, cnt in ((0, ntl), (1, nth)):
                        idx = idxlo if h == 0 else idxhi
                        view = lo_view if h == 0 else hi_view
                        done = 0
                        while done < cnt:
                            cn = min(CHUNK_TILES, cnt - done)
                            c0 = half_ofs[h] + done
                            vb = vpool.tile([128, cn, ES], f16, name="vb",
                                            tag="vb")
                            nc.gpsimd.dma_gather(
                                out_ap=vb[:], in_ap=view,
                                idxs_ap=idx[:, c0 * 8:(c0 + cn) * 8],
                                num_idxs=cn * 128, num_idxs_reg=cn * 128,
                                elem_size=ES,
                                queue_num=(callq[0] % 2))
                            callq[0] += 1
                            for i in range(cn):
                                gt = c0 + i + (0 if h == 0 else ntlo)
                                S = spool.tile([128, 128], f16, name="S",
                                               tag="S")
                                nc.vector.tensor_scalar(
                                    out=S[:], in0=iota[:],
                                    scalar1=colv[:, gt:gt + 1], scalar2=None,
                                    op0=Alu.is_equal)
                                nc.tensor.matmul(ps_b[:], lhsT=S[:],
                                                 rhs=vb[:, i, :D],
                                                 start=(ti == 0),
                                                 stop=(ti == ntot - 1))
                                ti += 1
                            done += cn
                    half_ofs[0] += ntl
                    half_ofs[1] += nth
                    if hop_i == 1:
                        nc.vector.tensor_scalar(
                            out=t1s[b][:], in0=ps_b[:],
                            scalar1=ndis_own[:, b:b + 1],
                            scalar2=None, op0=Alu.mult)
                        g = g1tiles[(b // GYB) % 2]
                        nc.vector.tensor_scalar(
                            out=g[:, b % GYB, :D], in0=t1s[b][:],
                            scalar1=dis_own[:, b:b + 1],
                            scalar2=None, op0=Alu.mult)
                        if b % GYB == GYB - 1:
                            gx_v = gx_in[:].rearrange("(n p) d -> p n d",
                                                      p=128)
                            nc.sync.dma_start(
                                out=gx_v[:, b - GYB + 1:b + 1, :], in_=g[:])
                    else:
                        t2 = xrot.tile([128, D], f32, name="t2", tag="t2")
                        nc.vector.tensor_scalar(
                            out=t2[:], in0=ps_b[:],
                            scalar1=ndis_own[:, b:b + 1],
                            scalar2=2.0, op0=Alu.mult, op1=Alu.mult)
                        nc.vector.tensor_tensor(
                            out=t2[:], in0=t2[:], in1=xown[b][:],
                            op=Alu.subtract)
                        phase_E(b, t2)

            # ---- hop 1 ----
            hop(1, g0_full[0:HALF, :], g0_full[HALF:NPAD, :])
            # ---- exchange T1 ----
            if SIM_SINGLE:
                nc.sync.dma_start(out=g1_full[0:CHUNK, :], in_=gx_in[:])
            else:
                nc.gpsimd.collective_compute(
                    "AllGather", Alu.bypass,
                    replica_groups=[list(range(NCORES))],
                    ins=[gx_in.opt()], outs=[g1_full.opt()])
            # ---- hop 2 (+ phase E per block) ----
            hop(2, g1_full[0:HALF, :], g1_full[HALF:NPAD, :])

            # ---- BN stats reduce + AllReduce ----
            st = work.tile([D, 2], f32)
            nc.vector.tensor_reduce(out=st[:, 0:1], in_=s1c[:],
                                    axis=mybir.AxisListType.X, op=Alu.add)
            nc.vector.tensor_reduce(out=st[:, 1:2], in_=s2c[:],
                                    axis=mybir.AxisListType.X, op=Alu.add)
            nc.sync.dma_start(out=st_in[:], in_=st[:])
            if SIM_SINGLE:
                nc.sync.dma_start(out=st_out[:], in_=st_in[:])
            else:
                nc.gpsimd.collective_compute(
                    "AllReduce", Alu.add,
                    replica_groups=[list(range(NCORES))],
                    ins=[st_in.opt()], outs=[st_out.opt()])
            gst = work.tile([D, 2], f32)
            nc.sync.dma_start(out=gst[:], in_=st_out[:])
            mean = work.tile([D, 1], f32)
            nc.vector.tensor_scalar(out=mean[:], in0=gst[:, 0:1],
                                    scalar1=1.0 / N, scalar2=None, op0=Alu.mult)
            var = work.tile([D, 1], f32)
            nc.vector.tensor_scalar(out=var[:], in0=gst[:, 1:2],
                                    scalar1=1.0 / N, scalar2=None, op0=Alu.mult)
            msq = work.tile([D, 1], f32)
            nc.vector.tensor_tensor(out=msq[:], in0=mean[:], in1=mean[:],
                                    op=Alu.mult)
            nc.vector.tensor_tensor(out=var[:], in0=var[:], in1=msq[:],
                                    op=Alu.subtract)
            nc.vector.tensor_scalar(out=var[:], in0=var[:], scalar1=EPS,
                                    scalar2=None, op0=Alu.add)
            sd = work.tile([D, 1], f32)
            nc.scalar.sqrt(sd[:], var[:])
            inv = work.tile([D, 1], f32)
            nc.vector.reciprocal(inv[:], sd[:])
            scl = work.tile([D, 1], f32)
            nc.vector.tensor_tensor(out=scl[:], in0=gam[:], in1=inv[:],
                                    op=Alu.mult)
            sh = work.tile([D, 1], f32)
            nc.vector.tensor_tensor(out=sh[:], in0=mean[:], in1=scl[:],
                                    op=Alu.mult)
            nc.vector.tensor_tensor(out=sh[:], in0=bet[:], in1=sh[:],
                                    op=Alu.subtract)

            # ---- normalize + emit (batched y writes, 7 blocks per DMA) ----
            yv = y_d.ap().rearrange("(n p) d -> p n d", p=128)
            for b in range(NBLK):
                fin = spool.tile([D, 128], f32, name="fin", tag="fin")
                nc.scalar.activation(out=fin[:], in_=outT[b][:],
                                     func=Act.Identity, bias=sh[:, 0:1],
                                     scale=scl[:, 0:1])
                tp2 = ptp.tile([128, D], f32, name="tp", tag="tp")
                nc.tensor.transpose(out=tp2[:], in_=fin[:],
                                    identity=ident[:D, :D])
                if b % GYB == 0:
                    yb = xrot.tile([128, GYB, D], f32, name="yb", tag="yb")
                nc.scalar.copy(yb[:, b % GYB, :], tp2[:])
                if b % GYB == GYB - 1:
                    nc.sync.dma_start(out=yv[:, b - GYB + 1:b + 1, :],
                                      in_=yb[:])

    nc.compile()
    return nc


def _prepare(x, edge_index, W, gamma, beta):
    x = np.asarray(x, np.float32)
    W = np.asarray(W, np.float32)
    gamma = np.asarray(gamma, np.float32).reshape(D, 1)
    beta = np.asarray(beta, np.float32).reshape(D, 1)

    T, per_core, deg_t, xpad = _preprocess(x, edge_index)

    key = T.tobytes()
    if key not in _cache:
        _cache[key] = _build(T)
    nc = _cache[key]

    in_maps = []
    for k in range(NCORES):
        pc = per_core[k]
        in_maps.append({
            "x": xpad, "x_own": pc["x_own"], "deg_t": deg_t,
            "deg_own": pc["deg_own"], "idx_lo": pc["idx_lo"],
            "idx_hi": pc["idx_hi"], "colv": pc["colv"],
            "W": W, "gamma": gamma, "beta": beta,
        })
    return nc, in_maps


def kernel(x, edge_index, W, bias, gamma, beta):
    from concourse.bass_utils import run_bass_kernel_spmd

    nc, in_maps = _prepare(x, edge_index, W, gamma, beta)
    res = run_bass_kernel_spmd(nc, in_maps, core_ids=list(range(NCORES)))
    y = np.concatenate([res.results[k]["y"] for k in range(NCORES)], axis=0)
    return np.ascontiguousarray(y[:N])
